# revision 19
# baseline (speedup 1.0000x reference)
"""Causal self-attention (B=4, T=2048, C=1024, H=16) on 8 trn2 NeuronCores.

Sharding: core c -> (batch b = c//2, head-group hg = c%2, 8 heads each).
Each core computes qkv projection for its (batch, head-group), causal
attention for its 8 heads, and a partial output projection. Host sums the
two partial y per batch (+ bout) and reassembles k, v.

Self-contained: hardcodes shapes; imports only installed libs (concourse).
"""

import numpy as np

B, T, C, H, D = 4, 2048, 1024, 16, 64
NHC = 8                # heads per core
HD = NHC * D           # 512 head-dims per core
SCALE = D ** -0.5
NCC = C // 128         # 8 contraction chunks for C
NM = HD // 128         # 4 head-pairs (m-tiles)
NJ = T // 512          # 4 query chunks
NTT = T // 128         # 16 t-tiles
N_CORES = 8

_CACHE = {}


def _build_program(dbg=False):
    import concourse.bass as bass
    from concourse import bacc, tile, mybir

    FP = mybir.dt.float32
    FPR = mybir.dt.float32r
    MMD = FPR  # matmul-path dtype
    EXP = mybir.ActivationFunctionType.Exp

    nc = bacc.Bacc("TRN2", debug=False, target_bir_lowering=False)

    xT = nc.dram_tensor("xT", [C, T], MMD, kind="ExternalInput")
    wq = nc.dram_tensor("wq", [C, HD], MMD, kind="ExternalInput")
    wk = nc.dram_tensor("wk", [C, HD], MMD, kind="ExternalInput")
    wv = nc.dram_tensor("wv", [C, HD], MMD, kind="ExternalInput")
    wo = nc.dram_tensor("wo", [HD, C], MMD, kind="ExternalInput")
    bq = nc.dram_tensor("bq", [HD], FP, kind="ExternalInput")
    bk = nc.dram_tensor("bk", [HD], FP, kind="ExternalInput")
    bvb = nc.dram_tensor("bvb", [128, HD], FP, kind="ExternalInput")
    maskin = nc.dram_tensor("maskin", [128, 896], MMD, kind="ExternalInput")
    y_o = nc.dram_tensor("y", [T, C], FP, kind="ExternalOutput")
    if dbg:
        qt_dbg = nc.dram_tensor("qt_dbg", [128, NM, T], FP, kind="ExternalOutput")
        ot_dbg = nc.dram_tensor("ot_dbg", [128, NM, T], FP, kind="ExternalOutput")
        pt_dbg = nc.dram_tensor("pt_dbg", [128, 1024], FP, kind="ExternalOutput")
        po_dbg = nc.dram_tensor("po_dbg", [128, 1024], FP, kind="ExternalOutput")
        rb_dbg = nc.dram_tensor("rb_dbg", [128, 512], FP, kind="ExternalOutput")
    kT_o = nc.dram_tensor("kT", [HD, T], FP, kind="ExternalOutput")
    v_o = nc.dram_tensor("v", [T, HD], FP, kind="ExternalOutput")

    def r(ap):
        return ap

    with tile.TileContext(nc) as tc:
        with tc.tile_pool(name="persist", bufs=1) as pp:
            QT = pp.tile([128, NM, T], MMD)
            KT = pp.tile([128, NM, T], MMD)
            VN = pp.tile([128, NTT, HD], MMD)
            bq_sb = pp.tile([128, NM], FP)
            bk_sb = pp.tile([128, NM], FP)
            bvb_sb = pp.tile([128, HD], FP)
            mask_sb = pp.tile([128, 896], MMD)
            ones_sb = pp.tile([128, 1], MMD)
            ones_f = pp.tile([128, 128], FP)

            nc.sync.dma_start(bq_sb[:], bq[:].rearrange("(m p) -> p m", p=128))
            nc.sync.dma_start(bk_sb[:], bk[:].rearrange("(m p) -> p m", p=128))
            nc.sync.dma_start(bvb_sb[:], bvb[:])
            nc.sync.dma_start(mask_sb[:], maskin[:])
            # memset can't write fp32r; stage fp32 ones and round via DVE copy
            nc.gpsimd.memset(ones_f[:], 1.0)
            nc.vector.tensor_copy(ones_sb[:], ones_f[:, 0:1])

            # ---------------- Phase 1: qkv projection ----------------
            with (
                tc.tile_pool(name="wsb", bufs=1) as wp,
                tc.tile_pool(name="xt", bufs=2) as xtp,
                tc.tile_pool(name="p1", bufs=4, space="PSUM") as p1,
            ):
                wq_sb = wp.tile([128, NCC, HD], MMD)
                wk_sb = wp.tile([128, NCC, HD], MMD)
                wv_sb = wp.tile([128, NCC, HD], MMD)
                nc.sync.dma_start(
                    wq_sb[:], wq[:].rearrange("(cc p) n -> p cc n", p=128))
                nc.sync.dma_start(
                    wk_sb[:], wk[:].rearrange("(cc p) n -> p cc n", p=128))
                nc.sync.dma_start(
                    wv_sb[:], wv[:].rearrange("(cc p) n -> p cc n", p=128))

                for tc5 in range(NJ):
                    ts = slice(tc5 * 512, tc5 * 512 + 512)
                    xt_t = xtp.tile([128, NCC, 512], MMD)
                    nc.sync.dma_start(
                        xt_t[:],
                        xT[:].rearrange("(cc p) t -> p cc t", p=128)[:, :, ts])
                    # Q^T and K^T m-tiles (head pairs)
                    for w_sb, b_sb, OUT in ((wq_sb, bq_sb, QT), (wk_sb, bk_sb, KT)):
                        for m in range(NM):
                            ps = p1.tile([128, 512], FP, tag="p1")
                            for cc in range(NCC):
                                nc.tensor.matmul(
                                    ps[:],
                                    r(w_sb[:, cc, m * 128:m * 128 + 128]),
                                    r(xt_t[:, cc, :]),
                                    start=(cc == 0), stop=(cc == NCC - 1))
                            nc.vector.tensor_scalar_add(
                                OUT[:, m, ts], ps[:], b_sb[:, m:m + 1])
                    # V natural t-slices
                    for st in range(4):
                        tt = tc5 * 4 + st
                        ps = p1.tile([128, 512], FP, tag="p1")
                        for cc in range(NCC):
                            nc.tensor.matmul(
                                ps[:],
                                r(xt_t[:, cc, st * 128:st * 128 + 128]),
                                r(wv_sb[:, cc, :]),
                                start=(cc == 0), stop=(cc == NCC - 1))
                        nc.vector.tensor_add(VN[:, tt, :], ps[:], bvb_sb[:])

            # k/v outputs (overlap with phase 2)
            nc.sync.dma_start(
                kT_o[:].rearrange("(m p) t -> p m t", p=128),
                KT[:].bitcast(FP))
            for tt in range(NTT):
                nc.sync.dma_start(
                    v_o[tt * 128:tt * 128 + 128, :].rearrange(
                        "p (h d) -> p h d", h=NHC),
                    VN[:, tt, :].rearrange("p (h d) -> p h d", h=NHC).bitcast(FP))

            # ---------------- Phase 2 + 3 ----------------
            with tc.tile_pool(name="ot", bufs=1) as otp:
                OT = otp.tile([128, NM, T], MMD)
                with (
                    tc.tile_pool(name="stp", bufs=2, space="PSUM") as stp,
                    tc.tile_pool(name="ops", bufs=1, space="PSUM") as ops,
                    tc.tile_pool(name="lps", bufs=1, space="PSUM") as lps,
                    tc.tile_pool(name="ptp", bufs=6) as ptp,
                    tc.tile_pool(name="recp", bufs=2) as recp,
                    tc.tile_pool(name="rbp", bufs=2) as rbp,
                    tc.tile_pool(name="dscr", bufs=2, space="DRAM") as dscr,
                ):
                    for hp in range(NM):
                        h0, h1 = 2 * hp, 2 * hp + 1
                        for j in range(NJ):
                            js = slice(j * 512, j * 512 + 512)
                            nkb = 4 * (j + 1)
                            # po: even head O^T at [0:64, 0:512] (bank A),
                            # odd head O^T at [0:64, 512:1024] (bank B).
                            # (fp32r matmul dst must start at partition 0.)
                            po = ops.tile([128, 1024], FP, tag="po")
                            # pl: softmax denominators, row 0 of two banks
                            pl = lps.tile([128, 1024], FP, tag="pl")
                            for kb in range(nkb):
                                ks = slice(kb * 128, kb * 128 + 128)
                                pst = stp.tile([128, 1024], FP, tag="pst")
                                nc.tensor.matmul(
                                    pst[:, 0:512],
                                    r(KT[0:64, hp, ks]), r(QT[0:64, hp, js]))
                                nc.tensor.matmul(
                                    pst[:, 512:1024],
                                    r(KT[64:128, hp, ks]), r(QT[64:128, hp, js]))
                                pt = ptp.tile([128, 1024], MMD, tag="pt")
                                nc.scalar.activation(
                                    pt[:], pst[:], EXP, scale=SCALE)
                                rband = kb - 4 * j
                                if rband >= 0:
                                    off = 384 - 128 * rband
                                    ms = mask_sb[:, off:off + 512]
                                    nc.vector.tensor_mul(
                                        pt[:, 0:512], pt[:, 0:512], ms)
                                    nc.vector.tensor_mul(
                                        pt[:, 512:1024], pt[:, 512:1024], ms)
                                mmkw = dict(start=(kb == 0), stop=(kb == nkb - 1))
                                nc.tensor.matmul(
                                    po[0:64, 0:512],
                                    r(VN[:, kb, h0 * 64:h0 * 64 + 64]),
                                    r(pt[:, 0:512]), **mmkw)
                                nc.tensor.matmul(
                                    po[0:64, 512:1024],
                                    r(VN[:, kb, h1 * 64:h1 * 64 + 64]),
                                    r(pt[:, 512:1024]), **mmkw)
                                nc.tensor.matmul(
                                    pl[0:1, 0:512], r(ones_sb[:, 0:1]),
                                    r(pt[:, 0:512]), **mmkw)
                                nc.tensor.matmul(
                                    pl[0:1, 512:1024], r(ones_sb[:, 0:1]),
                                    r(pt[:, 512:1024]), **mmkw)
                            # normalize: recip of l (row 0), DMA-broadcast to
                            # 64 partitions per head, then one aligned mul per
                            # head. No cross-partition compute ops.
                            ls = recp.tile([128, 2048], FP, tag="ls")
                            nc.vector.tensor_copy(ls[0:1, 0:1024], pl[0:1, :])
                            nc.vector.reciprocal(
                                ls[0:1, 1024:2048], ls[0:1, 0:1024])
                            # SBUF APs can't broadcast (zero partition step),
                            # so bounce the recip row through DRAM and
                            # broadcast on the way back.
                            scr = dscr.tile([1, 1024], FP, tag="scr")
                            nc.sync.dma_start(scr[:], ls[0:1, 1024:2048])
                            rb = rbp.tile([128, 512], FP, tag="rb")
                            nc.sync.dma_start(
                                rb[0:64, :],
                                scr[0:1, 0:512].to_broadcast((64, 512)))
                            nc.sync.dma_start(
                                rb[64:128, :],
                                scr[0:1, 512:1024].to_broadcast((64, 512)))
                            if dbg and hp == 0 and j == 0:
                                podt = ptp.tile([128, 1024], FP, tag="podbg")
                                nc.vector.tensor_copy(podt[:], po[:])
                                nc.sync.dma_start(po_dbg[:], podt[:])
                                nc.sync.dma_start(rb_dbg[:], rb[:])
                            nc.vector.tensor_mul(
                                OT[0:64, hp, js], po[0:64, 0:512], rb[0:64, :])
                            nc.vector.tensor_mul(
                                OT[64:128, hp, js], po[0:64, 512:1024],
                                rb[64:128, :])
                if dbg:
                    nc.sync.dma_start(qt_dbg[:], QT[:].bitcast(FP))
                    nc.sync.dma_start(ot_dbg[:], OT[:].bitcast(FP))

                # ---------------- Phase 3: output projection ----------------
                with (
                    tc.tile_pool(name="wop", bufs=1) as wop,
                    tc.tile_pool(name="p3", bufs=4, space="PSUM") as p3,
                    tc.tile_pool(name="ysb", bufs=3) as ysb,
                ):
                    wo_sb = wop.tile([128, NM, C], MMD)
                    nc.sync.dma_start(
                        wo_sb[:], wo[:].rearrange("(m p) c -> p m c", p=128))
                    for tt in range(NTT):
                        for cc2 in range(2):
                            ps = p3.tile([128, 512], FP, tag="p3")
                            for hp in range(NM):
                                nc.tensor.matmul(
                                    ps[:],
                                    r(OT[:, hp, tt * 128:tt * 128 + 128]),
                                    r(wo_sb[:, hp, cc2 * 512:cc2 * 512 + 512]),
                                    start=(hp == 0), stop=(hp == NM - 1))
                            yt = ysb.tile([128, 512], FP, tag="yt")
                            nc.vector.tensor_copy(yt[:], ps[:])
                            nc.sync.dma_start(
                                y_o[tt * 128:tt * 128 + 128,
                                    cc2 * 512:cc2 * 512 + 512], yt[:])

    nc.compile()
    return nc


def _get_program(dbg=False):
    key = ("nc", dbg)
    if key not in _CACHE:
        _CACHE[key] = _build_program(dbg)
    return _CACHE[key]


def _make_mask():
    # mask[p, g] = 1.0 iff g >= p + 384; slice [384-128r : 896-128r] gives the
    # causal 0/1 mask for a (128-key, 512-query) band block with offset r.
    p = np.arange(128)[:, None]
    g = np.arange(896)[None, :]
    return (g >= p + 384).astype(np.float32)


def _shard_inputs(x, Wqkv, bqkv, Wout):
    mask = _make_mask()
    xT_b = [np.ascontiguousarray(x[b].T) for b in range(B)]
    in_maps = []
    for c in range(N_CORES):
        b, hg = c // 2, c % 2
        cs = slice(hg * HD, hg * HD + HD)
        bv = bqkv[2 * C + hg * HD: 2 * C + hg * HD + HD]
        in_maps.append({
            "xT": xT_b[b],
            "wq": np.ascontiguousarray(Wqkv[:, hg * HD: hg * HD + HD]),
            "wk": np.ascontiguousarray(Wqkv[:, C + hg * HD: C + hg * HD + HD]),
            "wv": np.ascontiguousarray(Wqkv[:, 2 * C + hg * HD: 2 * C + hg * HD + HD]),
            "wo": np.ascontiguousarray(Wout[hg * HD: hg * HD + HD, :]),
            "bq": np.ascontiguousarray(bqkv[hg * HD: hg * HD + HD]),
            "bk": np.ascontiguousarray(bqkv[C + hg * HD: C + hg * HD + HD]),
            "bvb": np.ascontiguousarray(np.tile(bv, (128, 1))),
            "maskin": mask,
        })
    return in_maps


def _unshard(results, bqkv, bout):
    y = np.empty((B, T, C), dtype=np.float32)
    k = np.empty((B, H, T, D), dtype=np.float32)
    v = np.empty((B, H, T, D), dtype=np.float32)
    for c in range(N_CORES):
        b, hg = c // 2, c % 2
        res = results[c]
        hsl = slice(hg * NHC, hg * NHC + NHC)
        k[b, hsl] = res["kT"].reshape(NHC, D, T).transpose(0, 2, 1)
        v[b, hsl] = res["v"].reshape(T, NHC, D).transpose(1, 0, 2)
    for b in range(B):
        y[b] = results[2 * b]["y"] + results[2 * b + 1]["y"] + bout
    return y, k, v


LAST_EXEC_NS = None
LAST_RESULT = None
PROFILE = False
PROFILE_DIR = None


def kernel(x, Wqkv, bqkv, Wout, bout):
    global LAST_EXEC_NS, LAST_RESULT
    from concourse.bass_utils import run_bass_kernel_spmd

    x = np.asarray(x, dtype=np.float32)
    Wqkv = np.asarray(Wqkv, dtype=np.float32)
    bqkv = np.asarray(bqkv, dtype=np.float32)
    Wout = np.asarray(Wout, dtype=np.float32)
    bout = np.asarray(bout, dtype=np.float32)

    nc = _get_program()
    in_maps = _shard_inputs(x, Wqkv, bqkv, Wout)
    res = run_bass_kernel_spmd(
        nc, in_maps, list(range(N_CORES)), trace=PROFILE, tmpdir=PROFILE_DIR)
    LAST_EXEC_NS = res.exec_time_ns
    LAST_RESULT = res
    return _unshard(res.results, bqkv, bout)


# revision 21
# speedup vs baseline: 1.4288x; 1.4288x over previous
"""Causal self-attention (B=4, T=2048, C=1024, H=16) on 8 trn2 NeuronCores.

Sharding: core c -> (batch b = c//2, head-group hg = c%2, 8 heads each).
Each core computes the qkv projection for its (batch, head-group), causal
attention for its 8 heads, and a partial output projection. Host sums the
two partial y per batch (+ bout) and reassembles k, v.

Matmul path runs in float32r (tf32-grade, full PE rate at N>=512);
outputs carry fp32r-rounded values (~2.4e-4 rel err vs fp32 reference).

Self-contained: hardcodes shapes; imports only installed libs (concourse).
"""

import numpy as np

B, T, C, H, D = 4, 2048, 1024, 16, 64
NHC = 8                # heads per core
HD = NHC * D           # 512 head-dims per core
SCALE = D ** -0.5
NCC = C // 128         # 8 contraction chunks for C
NM = HD // 128         # 4 head-pairs (m-tiles)
NJ = T // 512          # 4 query chunks
NTT = T // 128         # 16 t-tiles
N_CORES = 8

_CACHE = {}


def _build_program(dbg=False):
    import concourse.bass as bass
    from concourse import bacc, tile, mybir

    FP = mybir.dt.float32
    MMD = mybir.dt.float32r
    EXP = mybir.ActivationFunctionType.Exp
    IDENT = mybir.ActivationFunctionType.Identity

    nc = bacc.Bacc("TRN2", debug=False, target_bir_lowering=False)

    xT = nc.dram_tensor("xT", [C, T], MMD, kind="ExternalInput")
    wq = nc.dram_tensor("wq", [C, HD], MMD, kind="ExternalInput")
    wk = nc.dram_tensor("wk", [C, HD], MMD, kind="ExternalInput")
    wv = nc.dram_tensor("wv", [C, HD], MMD, kind="ExternalInput")
    wo = nc.dram_tensor("wo", [HD, C], MMD, kind="ExternalInput")
    bq = nc.dram_tensor("bq", [HD], FP, kind="ExternalInput")
    bk = nc.dram_tensor("bk", [HD], FP, kind="ExternalInput")
    bvb = nc.dram_tensor("bvb", [128, HD], FP, kind="ExternalInput")
    maskin = nc.dram_tensor("maskin", [128, 128], MMD, kind="ExternalInput")
    y_o = nc.dram_tensor("y", [T, C], FP, kind="ExternalOutput")
    kT_o = nc.dram_tensor("kT", [HD, T], FP, kind="ExternalOutput")
    v_o = nc.dram_tensor("v", [T, HD], FP, kind="ExternalOutput")

    with tile.TileContext(nc) as tc:
        with (
            tc.tile_pool(name="persist", bufs=1) as pp,
            # two psum pools alive for the whole kernel: 4 + 4 banks
            tc.tile_pool(name="pstp", bufs=2, space="PSUM") as stp,
            tc.tile_pool(name="opsp", bufs=2, space="PSUM") as ops,
        ):
            QT = pp.tile([128, NM, T], MMD)
            KT = pp.tile([128, NM, T], MMD)
            # V natural; 65th column per head = 1.0 so the O^T matmul also
            # accumulates the softmax denominator l as psum row 64.
            VN = pp.tile([128, NTT, NHC, 65], MMD)
            bq_sb = pp.tile([128, NM], FP)
            bk_sb = pp.tile([128, NM], FP)
            bvb_sb = pp.tile([128, HD], FP)
            mask_sb = pp.tile([128, 128], MMD)
            ones_f = pp.tile([128, 128], FP)

            nc.sync.dma_start(bq_sb[:], bq[:].rearrange("(m p) -> p m", p=128))
            nc.sync.dma_start(bk_sb[:], bk[:].rearrange("(m p) -> p m", p=128))
            nc.sync.dma_start(bvb_sb[:], bvb[:])
            nc.sync.dma_start(mask_sb[:], maskin[:])
            # memset can't write fp32r; stage fp32 ones and round via DVE copy
            nc.gpsimd.memset(ones_f[:], 1.0)
            nc.vector.tensor_copy(
                VN[:, :, :, 64:65],
                ones_f[:].rearrange("p (a b c) -> p a b c", a=NTT, b=NHC))

            # ---------------- Phase 1: qkv projection ----------------
            with (
                tc.tile_pool(name="wsb", bufs=1) as wp,
                tc.tile_pool(name="xt", bufs=2) as xtp,
            ):
                wq_sb = wp.tile([128, NCC, HD], MMD)
                wk_sb = wp.tile([128, NCC, HD], MMD)
                wv_sb = wp.tile([128, NCC, HD], MMD)
                # per-cc chunks on the gpsimd queue so the first matmul only
                # waits for one 256 KB slice, and weights overlap x chunks
                for cc in range(NCC):
                    for wt, w_sb in ((wq, wq_sb), (wk, wk_sb), (wv, wv_sb)):
                        nc.gpsimd.dma_start(
                            w_sb[:, cc, :], wt[cc * 128:cc * 128 + 128, :])

                for tc5 in range(NJ):
                    ts = slice(tc5 * 512, tc5 * 512 + 512)
                    xt_t = xtp.tile([128, NCC, 512], MMD)
                    for cc in range(NCC):
                        nc.sync.dma_start(
                            xt_t[:, cc, :],
                            xT[cc * 128:cc * 128 + 128, ts])
                    # Q^T and K^T m-tiles (head pairs); psum pairs share a
                    # (128,1024) tile (two banks, two accumulation groups)
                    for w_sb, b_sb, OUT in ((wq_sb, bq_sb, QT),
                                            (wk_sb, bk_sb, KT)):
                        for mp in range(NM // 2):
                            ps = stp.tile([128, 1024], FP, tag="pst")
                            for half in range(2):
                                m = 2 * mp + half
                                hs = slice(half * 512, half * 512 + 512)
                                for cc in range(NCC):
                                    nc.tensor.matmul(
                                        ps[:, hs],
                                        w_sb[:, cc, m * 128:m * 128 + 128],
                                        xt_t[:, cc, :],
                                        start=(cc == 0), stop=(cc == NCC - 1))
                                # ACT is idle in phase 1: copy+bias there
                                nc.scalar.activation(
                                    OUT[:, m, ts], ps[:, hs], IDENT,
                                    bias=b_sb[:, m:m + 1])
                    # V natural t-slices (pairs share a psum tile)
                    for sp in range(2):
                        ps = ops.tile([128, 1024], FP, tag="po")
                        for half in range(2):
                            st = 2 * sp + half
                            tt = tc5 * 4 + st
                            hs = slice(half * 512, half * 512 + 512)
                            for cc in range(NCC):
                                nc.tensor.matmul(
                                    ps[:, hs],
                                    xt_t[:, cc, st * 128:st * 128 + 128],
                                    wv_sb[:, cc, :],
                                    start=(cc == 0), stop=(cc == NCC - 1))
                            nc.vector.tensor_add(
                                VN[:, tt, :, 0:64],
                                ps[:, hs].rearrange("p (h d) -> p h d", h=NHC),
                                bvb_sb[:].rearrange("p (h d) -> p h d", h=NHC))

            # k/v outputs on the vector-engine DMA queue (overlap phase 2)
            nc.scalar.dma_start(
                kT_o[:].rearrange("(m p) t -> p m t", p=128),
                KT[:].bitcast(FP))
            for tt in range(NTT):
                nc.scalar.dma_start(
                    v_o[tt * 128:tt * 128 + 128, :].rearrange(
                        "p (h d) -> p h d", h=NHC),
                    VN[:, tt, :, 0:64].bitcast(FP))

            # ---------------- Phase 2: attention ----------------
            with (
                tc.tile_pool(name="ot", bufs=1) as otp,
                tc.tile_pool(name="wop", bufs=1) as wop,
            ):
                OT = otp.tile([128, NM, T], MMD)
                wo_sb = wop.tile([128, NM, C], MMD)
                for cc in range(NM):
                    nc.gpsimd.dma_start(
                        wo_sb[:, cc, :], wo[cc * 128:cc * 128 + 128, :])
                with (
                    tc.tile_pool(name="ptp", bufs=6) as ptp,
                    tc.tile_pool(name="recp", bufs=3) as recp,
                    tc.tile_pool(name="rbp", bufs=3) as rbp,
                    tc.tile_pool(name="dscr", bufs=3, space="DRAM") as dscr,
                ):
                    for j in range(NJ):
                        js = slice(j * 512, j * 512 + 512)
                        nkb = 4 * (j + 1)
                        for hp in range(NM):
                            h0, h1 = 2 * hp, 2 * hp + 1
                            # po cols 0:512 even head, 512:1024 odd head;
                            # rows 0:64 = O^T, row 64 = softmax denominator
                            po = ops.tile([128, 1024], FP, tag="po")
                            for kb in range(nkb):
                                ks = slice(kb * 128, kb * 128 + 128)
                                pst = stp.tile([128, 1024], FP, tag="pst")
                                nc.tensor.matmul(
                                    pst[:, 0:512],
                                    KT[0:64, hp, ks], QT[0:64, hp, js])
                                nc.tensor.matmul(
                                    pst[:, 512:1024],
                                    KT[64:128, hp, ks], QT[64:128, hp, js])
                                pt = ptp.tile([128, 1024], MMD, tag="pt")
                                nc.scalar.activation(
                                    pt[:], pst[:], EXP, scale=SCALE)
                                rband = kb - 4 * j
                                lo = 0
                                if rband >= 0:
                                    # band block: columns [0,128r) are fully
                                    # masked -> skipped in the O matmul;
                                    # [128r,128r+128) get the triangular mask
                                    lo = 128 * rband
                                    for half in (0, 512):
                                        nc.vector.tensor_mul(
                                            pt[:, half + lo:half + lo + 128],
                                            pt[:, half + lo:half + lo + 128],
                                            mask_sb[:])
                                mmkw = dict(start=(kb == 0),
                                            stop=(kb == nkb - 1))
                                nc.tensor.matmul(
                                    po[0:65, lo:512],
                                    VN[:, kb, h0, :],
                                    pt[:, lo:512], **mmkw)
                                nc.tensor.matmul(
                                    po[0:65, 512 + lo:1024],
                                    VN[:, kb, h1, :],
                                    pt[:, 512 + lo:1024], **mmkw)
                            # normalize: 1/l then broadcast across partitions
                            # via a DRAM bounce (SBUF APs can't broadcast)
                            ls = recp.tile([128, 2048], FP, tag="ls")
                            nc.vector.tensor_copy(ls[0:1, 0:1024],
                                                  po[64:65, :])
                            nc.vector.reciprocal_approx_fast(
                                ls[0:1, 1024:2048], ls[0:1, 0:1024])
                            scr = dscr.tile([1, 1024], FP, tag="scr")
                            nc.gpsimd.dma_start(scr[:], ls[0:1, 1024:2048])
                            rb = rbp.tile([128, 512], FP, tag="rb")
                            nc.gpsimd.dma_start(
                                rb[0:64, :],
                                scr[0:1, 0:512].to_broadcast((64, 512)))
                            nc.gpsimd.dma_start(
                                rb[64:128, :],
                                scr[0:1, 512:1024].to_broadcast((64, 512)))
                            nc.vector.tensor_mul(
                                OT[0:64, hp, js], po[0:64, 0:512],
                                rb[0:64, :])
                            nc.vector.tensor_mul(
                                OT[64:128, hp, js], po[0:64, 512:1024],
                                rb[64:128, :])

                # ---------------- Phase 3: output projection ----------------
                with tc.tile_pool(name="ysb", bufs=4) as ysb:
                    for tt in range(NTT):
                        ps = stp.tile([128, 1024], FP, tag="pst")
                        for cc2 in range(2):
                            cs = slice(cc2 * 512, cc2 * 512 + 512)
                            for hp in range(NM):
                                nc.tensor.matmul(
                                    ps[:, cs],
                                    OT[:, hp, tt * 128:tt * 128 + 128],
                                    wo_sb[:, hp, cs],
                                    start=(hp == 0), stop=(hp == NM - 1))
                        yt = ysb.tile([128, 1024], FP, tag="yt")
                        nc.scalar.activation(yt[:], ps[:], IDENT)
                        nc.sync.dma_start(
                            y_o[tt * 128:tt * 128 + 128, :], yt[:])

    nc.compile()
    return nc


def _get_program(dbg=False):
    key = ("nc", dbg)
    if key not in _CACHE:
        _CACHE[key] = _build_program(dbg)
    return _CACHE[key]


def _make_mask():
    # (128,128) lower-triangular 0/1: mask[p, g] = 1.0 iff g >= p
    p = np.arange(128)[:, None]
    g = np.arange(128)[None, :]
    return (g >= p).astype(np.float32)


def _shard_inputs(x, Wqkv, bqkv, Wout):
    mask = _make_mask()
    xT_b = [np.ascontiguousarray(x[b].T) for b in range(B)]
    in_maps = []
    for c in range(N_CORES):
        b, hg = c // 2, c % 2
        bv = bqkv[2 * C + hg * HD: 2 * C + hg * HD + HD]
        in_maps.append({
            "xT": xT_b[b],
            "wq": np.ascontiguousarray(Wqkv[:, hg * HD: hg * HD + HD]),
            "wk": np.ascontiguousarray(Wqkv[:, C + hg * HD: C + hg * HD + HD]),
            "wv": np.ascontiguousarray(Wqkv[:, 2 * C + hg * HD: 2 * C + hg * HD + HD]),
            "wo": np.ascontiguousarray(Wout[hg * HD: hg * HD + HD, :]),
            "bq": np.ascontiguousarray(bqkv[hg * HD: hg * HD + HD]),
            "bk": np.ascontiguousarray(bqkv[C + hg * HD: C + hg * HD + HD]),
            "bvb": np.ascontiguousarray(np.tile(bv, (128, 1))),
            "maskin": mask,
        })
    return in_maps


def _unshard(results, bqkv, bout):
    y = np.empty((B, T, C), dtype=np.float32)
    k = np.empty((B, H, T, D), dtype=np.float32)
    v = np.empty((B, H, T, D), dtype=np.float32)
    for c in range(N_CORES):
        b, hg = c // 2, c % 2
        res = results[c]
        hsl = slice(hg * NHC, hg * NHC + NHC)
        k[b, hsl] = res["kT"].reshape(NHC, D, T).transpose(0, 2, 1)
        v[b, hsl] = res["v"].reshape(T, NHC, D).transpose(1, 0, 2)
    for b in range(B):
        y[b] = results[2 * b]["y"] + results[2 * b + 1]["y"] + bout
    return y, k, v


LAST_EXEC_NS = None
LAST_RESULT = None
PROFILE = False
PROFILE_DIR = None


def kernel(x, Wqkv, bqkv, Wout, bout):
    global LAST_EXEC_NS, LAST_RESULT
    from concourse.bass_utils import run_bass_kernel_spmd

    x = np.asarray(x, dtype=np.float32)
    Wqkv = np.asarray(Wqkv, dtype=np.float32)
    bqkv = np.asarray(bqkv, dtype=np.float32)
    Wout = np.asarray(Wout, dtype=np.float32)
    bout = np.asarray(bout, dtype=np.float32)

    nc = _get_program()
    in_maps = _shard_inputs(x, Wqkv, bqkv, Wout)
    res = run_bass_kernel_spmd(
        nc, in_maps, list(range(N_CORES)), trace=PROFILE, tmpdir=PROFILE_DIR)
    LAST_EXEC_NS = res.exec_time_ns
    LAST_RESULT = res
    return _unshard(res.results, bqkv, bout)


# revision 23
# speedup vs baseline: 1.7379x; 1.2163x over previous
"""Causal self-attention (B=4, T=2048, C=1024, H=16) on 8 trn2 NeuronCores.

Sharding: core c -> (batch b = c//2, head-group hg = c%2, 8 heads each).
Each core computes the qkv projection for its (batch, head-group), causal
attention for its 8 heads, and a partial output projection. Host sums the
two partial y per batch (+ bout) and reassembles k, v.

Matmul path runs in float32r (tf32-grade, full PE rate at N>=512);
outputs carry fp32r-rounded values (~2.4e-4 rel err vs fp32 reference).

Self-contained: hardcodes shapes; imports only installed libs (concourse).
"""

import numpy as np

B, T, C, H, D = 4, 2048, 1024, 16, 64
NHC = 8                # heads per core
HD = NHC * D           # 512 head-dims per core
SCALE = D ** -0.5
NCC = C // 128         # 8 contraction chunks for C
NM = HD // 128         # 4 head-pairs (m-tiles)
NJ = T // 512          # 4 query chunks
NTT = T // 128         # 16 t-tiles
N_CORES = 8

_CACHE = {}


def _build_program(dbg=False):
    import concourse.bass as bass
    from concourse import bacc, tile, mybir

    FP = mybir.dt.float32
    MMD = mybir.dt.float32r
    EXP = mybir.ActivationFunctionType.Exp
    IDENT = mybir.ActivationFunctionType.Identity

    nc = bacc.Bacc("TRN2", debug=False, target_bir_lowering=False)

    xT = nc.dram_tensor("xT", [C, T], MMD, kind="ExternalInput")
    wq = nc.dram_tensor("wq", [C, HD], MMD, kind="ExternalInput")
    wk = nc.dram_tensor("wk", [C, HD], MMD, kind="ExternalInput")
    wv = nc.dram_tensor("wv", [C, HD], MMD, kind="ExternalInput")
    wo = nc.dram_tensor("wo", [HD, C], MMD, kind="ExternalInput")
    bq = nc.dram_tensor("bq", [HD], FP, kind="ExternalInput")
    bk = nc.dram_tensor("bk", [HD], FP, kind="ExternalInput")
    bvb = nc.dram_tensor("bvb", [128, HD], FP, kind="ExternalInput")
    maskin = nc.dram_tensor("maskin", [128, 128], MMD, kind="ExternalInput")
    y_o = nc.dram_tensor("y", [T, C], FP, kind="ExternalOutput")
    kT_o = nc.dram_tensor("kT", [HD, T], FP, kind="ExternalOutput")
    v_o = nc.dram_tensor("v", [T, HD], FP, kind="ExternalOutput")

    with tile.TileContext(nc) as tc:
        with (
            tc.tile_pool(name="persist", bufs=1) as pp,
            # two psum pools alive for the whole kernel: 4 + 4 banks
            tc.tile_pool(name="pstp", bufs=2, space="PSUM") as stp,
            tc.tile_pool(name="opsp", bufs=2, space="PSUM") as ops,
        ):
            # per-t-chunk tiles so Tile's per-tile dep tracking lets
            # phase 2 start on chunk 0 while phase 1 still computes chunk 1+
            QT = [pp.tile([128, NM, 512], MMD, tag=f"qt{i}", name=f"qt{i}")
                  for i in range(NJ)]
            KT = [pp.tile([128, NM, 512], MMD, tag=f"kt{i}", name=f"kt{i}")
                  for i in range(NJ)]
            # V natural; 65th column per head = 1.0 so the O^T matmul also
            # accumulates the softmax denominator l as psum row 64.
            VN = [pp.tile([128, 4, NHC, 65], MMD, tag=f"vn{i}", name=f"vn{i}")
                  for i in range(NJ)]
            bq_sb = pp.tile([128, NM], FP)
            bk_sb = pp.tile([128, NM], FP)
            bvb_sb = pp.tile([128, HD], FP)
            mask_sb = pp.tile([128, 128], MMD)
            ones_f = pp.tile([128, 128], FP)

            nc.sync.dma_start(bq_sb[:], bq[:].rearrange("(m p) -> p m", p=128))
            nc.sync.dma_start(bk_sb[:], bk[:].rearrange("(m p) -> p m", p=128))
            nc.sync.dma_start(bvb_sb[:], bvb[:])
            nc.sync.dma_start(mask_sb[:], maskin[:])
            # memset can't write fp32r; stage fp32 ones and round via DVE copy
            nc.gpsimd.memset(ones_f[:], 1.0)
            for i in range(NJ):
                nc.vector.tensor_copy(
                    VN[i][:, :, :, 64:65],
                    ones_f[:, 0:32].rearrange(
                        "p (a b c) -> p a b c", a=4, b=NHC))

            # ---------------- Phase 1: qkv projection ----------------
            with (
                tc.tile_pool(name="wsb", bufs=1) as wp,
                tc.tile_pool(name="xt", bufs=2) as xtp,
            ):
                wq_sb = wp.tile([128, NCC, HD], MMD)
                wk_sb = wp.tile([128, NCC, HD], MMD)
                wv_sb = wp.tile([128, NCC, HD], MMD)
                # per-cc chunks on the gpsimd queue so the first matmul only
                # waits for one 256 KB slice, and weights overlap x chunks
                for cc in range(NCC):
                    for wt, w_sb in ((wq, wq_sb), (wk, wk_sb), (wv, wv_sb)):
                        nc.gpsimd.dma_start(
                            w_sb[:, cc, :], wt[cc * 128:cc * 128 + 128, :])

                for tc5 in range(NJ):
                    ts = slice(tc5 * 512, tc5 * 512 + 512)
                    xt_t = xtp.tile([128, NCC, 512], MMD)
                    for cc in range(NCC):
                        nc.sync.dma_start(
                            xt_t[:, cc, :],
                            xT[cc * 128:cc * 128 + 128, ts])
                    # Q^T and K^T m-tiles (head pairs); psum pairs share a
                    # (128,1024) tile (two banks, two accumulation groups)
                    for w_sb, b_sb, OUT in ((wq_sb, bq_sb, QT),
                                            (wk_sb, bk_sb, KT)):
                        for mp in range(NM // 2):
                            ps = stp.tile([128, 1024], FP, tag="pst")
                            for half in range(2):
                                m = 2 * mp + half
                                hs = slice(half * 512, half * 512 + 512)
                                for cc in range(NCC):
                                    nc.tensor.matmul(
                                        ps[:, hs],
                                        w_sb[:, cc, m * 128:m * 128 + 128],
                                        xt_t[:, cc, :],
                                        start=(cc == 0), stop=(cc == NCC - 1))
                                nc.vector.tensor_scalar_add(
                                    OUT[tc5][:, m, :], ps[:, hs],
                                    b_sb[:, m:m + 1])
                    # V natural t-slices (pairs share a psum tile)
                    for sp in range(2):
                        ps = ops.tile([128, 1024], FP, tag="po")
                        for half in range(2):
                            st = 2 * sp + half
                            tt = tc5 * 4 + st
                            hs = slice(half * 512, half * 512 + 512)
                            for cc in range(NCC):
                                nc.tensor.matmul(
                                    ps[:, hs],
                                    xt_t[:, cc, st * 128:st * 128 + 128],
                                    wv_sb[:, cc, :],
                                    start=(cc == 0), stop=(cc == NCC - 1))
                            nc.vector.tensor_add(
                                VN[tc5][:, st, :, 0:64],
                                ps[:, hs].rearrange("p (h d) -> p h d", h=NHC),
                                bvb_sb[:].rearrange("p (h d) -> p h d", h=NHC))

            # k/v outputs (per chunk; overlap with phase 2)
            for i in range(NJ):
                nc.sync.dma_start(
                    kT_o[:, i * 512:i * 512 + 512].rearrange(
                        "(m p) t -> p m t", p=128),
                    KT[i][:].bitcast(FP))
                for st in range(4):
                    tt = i * 4 + st
                    nc.sync.dma_start(
                        v_o[tt * 128:tt * 128 + 128, :].rearrange(
                            "p (h d) -> p h d", h=NHC),
                        VN[i][:, st, :, 0:64].bitcast(FP))

            # ---------------- Phase 2: attention ----------------
            with (
                tc.tile_pool(name="ot", bufs=1) as otp,
                tc.tile_pool(name="wop", bufs=1) as wop,
            ):
                OT = [otp.tile([128, NM, 512], MMD, tag=f"ot{i}", name=f"ot{i}")
                      for i in range(NJ)]
                wo_sb = wop.tile([128, NM, C], MMD)
                for cc in range(NM):
                    nc.gpsimd.dma_start(
                        wo_sb[:, cc, :], wo[cc * 128:cc * 128 + 128, :])
                with (
                    tc.tile_pool(name="ptp", bufs=6) as ptp,
                    tc.tile_pool(name="recp", bufs=3) as recp,
                    tc.tile_pool(name="rbp", bufs=3) as rbp,
                    tc.tile_pool(name="dscr", bufs=3, space="DRAM") as dscr,
                ):
                    for j in range(NJ):
                        js = slice(j * 512, j * 512 + 512)
                        nkb = 4 * (j + 1)
                        for hp in range(NM):
                            h0, h1 = 2 * hp, 2 * hp + 1
                            # po cols 0:512 even head, 512:1024 odd head;
                            # rows 0:64 = O^T, row 64 = softmax denominator
                            po = ops.tile([128, 1024], FP, tag="po")
                            for kb in range(nkb):
                                ks = slice(kb * 128, kb * 128 + 128)
                                kc, ko = kb // 4, (kb % 4) * 128
                                pst = stp.tile([128, 1024], FP, tag="pst")
                                nc.tensor.matmul(
                                    pst[:, 0:512],
                                    KT[kc][0:64, hp, ko:ko + 128],
                                    QT[j][0:64, hp, :])
                                nc.tensor.matmul(
                                    pst[:, 512:1024],
                                    KT[kc][64:128, hp, ko:ko + 128],
                                    QT[j][64:128, hp, :])
                                pt = ptp.tile([128, 1024], MMD, tag="pt")
                                nc.scalar.activation(
                                    pt[:], pst[:], EXP, scale=SCALE)
                                rband = kb - 4 * j
                                lo = 0
                                if rband >= 0:
                                    # band block: columns [0,128r) are fully
                                    # masked -> skipped in the O matmul;
                                    # [128r,128r+128) get the triangular mask
                                    lo = 128 * rband
                                    for half in (0, 512):
                                        nc.vector.tensor_mul(
                                            pt[:, half + lo:half + lo + 128],
                                            pt[:, half + lo:half + lo + 128],
                                            mask_sb[:])
                                mmkw = dict(start=(kb == 0),
                                            stop=(kb == nkb - 1))
                                nc.tensor.matmul(
                                    po[0:65, lo:512],
                                    VN[kc][:, kb % 4, h0, :],
                                    pt[:, lo:512], **mmkw)
                                nc.tensor.matmul(
                                    po[0:65, 512 + lo:1024],
                                    VN[kc][:, kb % 4, h1, :],
                                    pt[:, 512 + lo:1024], **mmkw)
                            # normalize: 1/l then broadcast across partitions
                            # via a DRAM bounce (SBUF APs can't broadcast)
                            ls = recp.tile([128, 2048], FP, tag="ls")
                            nc.vector.tensor_copy(ls[0:1, 0:1024],
                                                  po[64:65, :])
                            nc.vector.reciprocal_approx_fast(
                                ls[0:1, 1024:2048], ls[0:1, 0:1024])
                            scr = dscr.tile([1, 1024], FP, tag="scr")
                            nc.gpsimd.dma_start(scr[:], ls[0:1, 1024:2048])
                            rb = rbp.tile([128, 512], FP, tag="rb")
                            nc.gpsimd.dma_start(
                                rb[0:64, :],
                                scr[0:1, 0:512].to_broadcast((64, 512)))
                            nc.gpsimd.dma_start(
                                rb[64:128, :],
                                scr[0:1, 512:1024].to_broadcast((64, 512)))
                            nc.vector.tensor_mul(
                                OT[j][0:64, hp, :], po[0:64, 0:512],
                                rb[0:64, :])
                            nc.vector.tensor_mul(
                                OT[j][64:128, hp, :], po[0:64, 512:1024],
                                rb[64:128, :])

                # ---------------- Phase 3: output projection ----------------
                with tc.tile_pool(name="ysb", bufs=4) as ysb:
                    for tt in range(NTT):
                        oc, oo = tt // 4, (tt % 4) * 128
                        ps = stp.tile([128, 1024], FP, tag="pst")
                        for cc2 in range(2):
                            cs = slice(cc2 * 512, cc2 * 512 + 512)
                            for hp in range(NM):
                                nc.tensor.matmul(
                                    ps[:, cs],
                                    OT[oc][:, hp, oo:oo + 128],
                                    wo_sb[:, hp, cs],
                                    start=(hp == 0), stop=(hp == NM - 1))
                        yt = ysb.tile([128, 1024], FP, tag="yt")
                        nc.vector.tensor_copy(yt[:], ps[:])
                        nc.sync.dma_start(
                            y_o[tt * 128:tt * 128 + 128, :], yt[:])

    nc.compile()
    return nc


def _get_program(dbg=False):
    key = ("nc", dbg)
    if key not in _CACHE:
        _CACHE[key] = _build_program(dbg)
    return _CACHE[key]


def _make_mask():
    # (128,128) lower-triangular 0/1: mask[p, g] = 1.0 iff g >= p
    p = np.arange(128)[:, None]
    g = np.arange(128)[None, :]
    return (g >= p).astype(np.float32)


def _shard_inputs(x, Wqkv, bqkv, Wout):
    mask = _make_mask()
    xT_b = [np.ascontiguousarray(x[b].T) for b in range(B)]
    in_maps = []
    for c in range(N_CORES):
        b, hg = c // 2, c % 2
        bv = bqkv[2 * C + hg * HD: 2 * C + hg * HD + HD]
        in_maps.append({
            "xT": xT_b[b],
            "wq": np.ascontiguousarray(Wqkv[:, hg * HD: hg * HD + HD]),
            "wk": np.ascontiguousarray(Wqkv[:, C + hg * HD: C + hg * HD + HD]),
            "wv": np.ascontiguousarray(Wqkv[:, 2 * C + hg * HD: 2 * C + hg * HD + HD]),
            "wo": np.ascontiguousarray(Wout[hg * HD: hg * HD + HD, :]),
            "bq": np.ascontiguousarray(bqkv[hg * HD: hg * HD + HD]),
            "bk": np.ascontiguousarray(bqkv[C + hg * HD: C + hg * HD + HD]),
            "bvb": np.ascontiguousarray(np.tile(bv, (128, 1))),
            "maskin": mask,
        })
    return in_maps


def _unshard(results, bqkv, bout):
    y = np.empty((B, T, C), dtype=np.float32)
    k = np.empty((B, H, T, D), dtype=np.float32)
    v = np.empty((B, H, T, D), dtype=np.float32)
    for c in range(N_CORES):
        b, hg = c // 2, c % 2
        res = results[c]
        hsl = slice(hg * NHC, hg * NHC + NHC)
        k[b, hsl] = res["kT"].reshape(NHC, D, T).transpose(0, 2, 1)
        v[b, hsl] = res["v"].reshape(T, NHC, D).transpose(1, 0, 2)
    for b in range(B):
        y[b] = results[2 * b]["y"] + results[2 * b + 1]["y"] + bout
    return y, k, v


LAST_EXEC_NS = None
LAST_RESULT = None
PROFILE = False
PROFILE_DIR = None


def kernel(x, Wqkv, bqkv, Wout, bout):
    global LAST_EXEC_NS, LAST_RESULT
    from concourse.bass_utils import run_bass_kernel_spmd

    x = np.asarray(x, dtype=np.float32)
    Wqkv = np.asarray(Wqkv, dtype=np.float32)
    bqkv = np.asarray(bqkv, dtype=np.float32)
    Wout = np.asarray(Wout, dtype=np.float32)
    bout = np.asarray(bout, dtype=np.float32)

    nc = _get_program()
    in_maps = _shard_inputs(x, Wqkv, bqkv, Wout)
    res = run_bass_kernel_spmd(
        nc, in_maps, list(range(N_CORES)), trace=PROFILE, tmpdir=PROFILE_DIR)
    LAST_EXEC_NS = res.exec_time_ns
    LAST_RESULT = res
    return _unshard(res.results, bqkv, bout)
